# revision 16
# baseline (speedup 1.0000x reference)
"""GQA attention kernel for Trainium2, 8 NeuronCores.

Problem: resid [2, 2048, 1024], 16 Q heads / 8 KV groups, d_head 64, causal,
out = softmax(QK^T/8 + causal) V -> W_out + b_out.

Sharding: tensor-parallel over (batch x kv-group-pairs). Core c handles
batch b = c // 4 and kv groups {2*(c%4), 2*(c%4)+1} = 4 Q heads. Each core
computes its heads' attention and a partial output projection; the host sums
the 4 partials per batch element and adds b_out.

Per-core dataflow (fp32 storage, float32r matmuls = full PE speed at
moving-dim >= 256):
  - host passes resid[b].T so the d_model contraction lands on partitions
  - Q^T [256, S] and K^T [128, S] projections (PSUM accum over 8 d-chunks)
  - V [S, 2x65] with a ones column appended per group -> the AV matmul
    produces sum-exp for free in output row 64
  - scores computed transposed: S^T[k, q] = K @ Q^T; causality via q-start
    offset, zero-padding of exp tiles, and an upper-triangular
    multiplicative mask on diagonal tiles
  - softmax without max-subtraction (scores are O(1) by construction;
    masked lanes are exactly zero after the mask multiply)
  - U^T[e, q] += V_aug^T @ exp accumulated over k-tiles in PSUM
  - normalize: reciprocal of row 64 (VectorE), partition-broadcast
    (GpSimd), multiply into z^T (VectorE)
  - out_partial[s, d] = z^T.T @ W_out_stack accumulated over 2 e-chunks
"""

import sys

sys.path.insert(0, "/opt/trn_rl_repo")

import numpy as np

import concourse.bass as bass
import concourse.mybir as mybir
import concourse.tile as tile
from concourse import bacc
from concourse.bass_utils import run_bass_kernel_spmd
from concourse.masks import make_upper_triangular

S = 2048          # seq len
D = 1024          # d_model
E = 64            # d_head
P = 128
NC_HEADS = 4      # heads per core
NCHUNK = D // P   # 8 d_model chunks
SPAN = 512
NSPAN = S // SPAN
NKT = S // P      # 16 k tiles
F32 = mybir.dt.float32
F32R = mybir.dt.float32r
EXP = mybir.ActivationFunctionType.Exp

LAST_RESULTS = None  # stashed BassKernelResults for the test harness
_CACHED_NC = None


def _build_program():
    nc = bacc.Bacc("TRN2", target_bir_lowering=False, debug=False)

    rT_d = nc.dram_tensor("resid_t", [D, S], F32R, kind="ExternalInput")
    wq_d = nc.dram_tensor("wq", [D, 256], F32R, kind="ExternalInput")
    wk_d = nc.dram_tensor("wk", [D, 128], F32R, kind="ExternalInput")
    wv_d = nc.dram_tensor("wv", [D, 128], F32R, kind="ExternalInput")
    wo_d = nc.dram_tensor("wo", [256, D], F32R, kind="ExternalInput")
    ones_d = nc.dram_tensor("ones", [P, 1], F32R, kind="ExternalInput")
    out_d = nc.dram_tensor("out", [S, D], F32, kind="ExternalOutput")

    with tile.TileContext(nc) as tc:
        with (
            tc.tile_pool(name="persist", bufs=1) as pp,
            tc.tile_pool(name="exp", bufs=6) as ep,
            tc.tile_pool(name="zt", bufs=2) as zp,
            tc.tile_pool(name="misc", bufs=4) as mp,
            tc.tile_pool(name="ostage", bufs=3) as op,
            tc.tile_pool(name="ps_u", bufs=4, space="PSUM") as ps_u,
            tc.tile_pool(name="ps_sc", bufs=2, space="PSUM") as ps_sc,
            tc.tile_pool(name="ps_op", bufs=2, space="PSUM") as ps_op,
        ):
            # ---- load weights + transposed residual ----
            rT = []
            for c in range(NCHUNK):
                t = pp.tile([P, S], F32R, tag=f"rt{c}")
                nc.sync.dma_start(t[:], rT_d[c * P:(c + 1) * P, :])
                rT.append(t)
            wq_sb = []
            wk_sb = []
            wv_sb = []
            for c in range(NCHUNK):
                t = pp.tile([P, 256], F32R, tag=f"wq{c}")
                nc.sync.dma_start(t[:], wq_d[c * P:(c + 1) * P, :])
                wq_sb.append(t)
                t = pp.tile([P, 128], F32R, tag=f"wk{c}")
                nc.sync.dma_start(t[:], wk_d[c * P:(c + 1) * P, :])
                wk_sb.append(t)
                t = pp.tile([P, 128], F32R, tag=f"wv{c}")
                nc.sync.dma_start(t[:], wv_d[c * P:(c + 1) * P, :])
                wv_sb.append(t)
            wo_sb = []
            for c in range(2):
                t = pp.tile([P, D], F32R, tag=f"wo{c}")
                nc.sync.dma_start(t[:], wo_d[c * P:(c + 1) * P, :])
                wo_sb.append(t)

            mask = pp.tile([P, P], F32, tag="mask")
            make_upper_triangular(nc, mask[:], val=1.0, diag=True)

            # ---- Q^T projection: qT[eblk] [128, S], eblk 0 = heads 0,1 ----
            qT = []
            for eblk in range(2):
                qt = pp.tile([P, S], F32R, tag=f"qt{eblk}")
                qT.append(qt)
                for sp in range(NSPAN):
                    acc = ps_u.tile([P, SPAN], F32, tag="u")
                    for c in range(NCHUNK):
                        nc.tensor.matmul(
                            acc[:],
                            wq_sb[c][:, eblk * P:(eblk + 1) * P],
                            rT[c][:, sp * SPAN:(sp + 1) * SPAN],
                            start=(c == 0),
                            stop=(c == NCHUNK - 1),
                        )
                    nc.scalar.copy(qt[:, sp * SPAN:(sp + 1) * SPAN], acc[:])

            # ---- K^T projection: kT [128, S] (rows = 2 groups x 64) ----
            kT = pp.tile([P, S], F32R, tag="kt")
            for sp in range(NSPAN):
                acc = ps_u.tile([P, SPAN], F32, tag="u")
                for c in range(NCHUNK):
                    nc.tensor.matmul(
                        acc[:],
                        wk_sb[c][:],
                        rT[c][:, sp * SPAN:(sp + 1) * SPAN],
                        start=(c == 0),
                        stop=(c == NCHUNK - 1),
                    )
                nc.scalar.copy(kT[:, sp * SPAN:(sp + 1) * SPAN], acc[:])

            # ---- V projection + ones column: vaug[kt] [128, 130] ----
            vaug = []
            for kt in range(NKT):
                va = pp.tile([P, 130], F32R, tag=f"va{kt}")
                vaug.append(va)
                acc = ps_sc.tile([P, SPAN], F32, tag="sc")
                for c in range(NCHUNK):
                    nc.tensor.matmul(
                        acc[:, 0:128],
                        rT[c][:, kt * P:(kt + 1) * P],
                        wv_sb[c][:],
                        start=(c == 0),
                        stop=(c == NCHUNK - 1),
                    )
                nc.vector.tensor_copy(va[:, 0:64], acc[:, 0:64])
                nc.vector.tensor_copy(va[:, 65:129], acc[:, 64:128])
                nc.sync.dma_start(va[:, 64:65], ones_d[:])
                nc.sync.dma_start(va[:, 129:130], ones_d[:])

            # ---- attention + output projection, span by span ----
            for sp in range(NSPAN):
                q0 = sp * SPAN
                nkt = (q0 + SPAN) // P  # k tiles touching this span
                # head slot (g, i): local head 2g+i, stored in qT[i] rows
                # g*64:(g+1)*64 so scores lhsT/rhs share base partition g*64
                # (and g0/g1 matmuls row-pack the PE array).
                u_ps = [ps_u.tile([P, SPAN], F32, tag="u", name=f"u{j}")
                        for j in range(NC_HEADS)]
                for kt in range(nkt):
                    k0 = kt * P
                    off = max(k0 - q0, 0)
                    w = SPAN - off
                    for g in range(2):
                        for i in range(2):
                            s_ps = ps_sc.tile([P, SPAN], F32, tag="sc",
                                              name=f"s{g}{i}")
                            nc.tensor.matmul(
                                s_ps[:, off:off + w],
                                kT[g * 64:(g + 1) * 64, k0:k0 + P],
                                qT[i][g * 64:(g + 1) * 64,
                                         q0 + off:q0 + off + w],
                                start=True,
                                stop=True,
                            )
                            e_sb = ep.tile([P, SPAN], F32R, tag="e",
                                           name=f"e{g}{i}")
                            nc.scalar.activation(
                                e_sb[:, off:off + w], s_ps[:, off:off + w],
                                EXP, scale=0.125,
                            )
                            if k0 >= q0:  # diagonal tile -> causal mask
                                nc.vector.tensor_mul(
                                    e_sb[:, off:off + P],
                                    e_sb[:, off:off + P].bitcast(F32),
                                    mask[:],
                                )
                            # partial-width accumulate: cols < off keep
                            # earlier k-tiles' sums (kt==0 covers full span)
                            nc.tensor.matmul(
                                u_ps[2 * g + i][0:65, off:off + w],
                                vaug[kt][:, g * 65:(g + 1) * 65],
                                e_sb[:, off:off + w],
                                start=(kt == 0),
                                stop=(kt == nkt - 1),
                                skip_group_check=True,
                            )

                # normalize -> z^T chunks; zc[i] rows g*64 = head slot (g, i),
                # matching the host-side wo packing [h0, h2 | h1, h3]
                zc = [zp.tile([P, SPAN], F32R, tag=f"zt{c}", name=f"z{c}")
                      for c in range(2)]
                for g in range(2):
                    for i in range(2):
                        rec = mp.tile([1, SPAN], F32, tag="rec", name="rec")
                        nc.vector.reciprocal(rec[:], u_ps[2 * g + i][64:65, :])
                        bc = mp.tile([64, SPAN], F32, tag="bc", name="bc")
                        nc.gpsimd.partition_broadcast(bc[:], rec[:])
                        nc.vector.tensor_mul(
                            zc[i][g * 64:(g + 1) * 64, :],
                            u_ps[2 * g + i][0:64, :],
                            bc[:],
                        )

                # output projection for this span of s
                for st in range(4):
                    s0 = q0 + st * P
                    o_sb = op.tile([P, D], F32, tag="ost")
                    for dsp in range(2):
                        o_ps = ps_op.tile([P, SPAN], F32, tag="op")
                        for ch in range(2):
                            nc.tensor.matmul(
                                o_ps[:],
                                zc[ch][:, st * P:(st + 1) * P],
                                wo_sb[ch][:, dsp * SPAN:(dsp + 1) * SPAN],
                                start=(ch == 0),
                                stop=(ch == 1),
                            )
                        if (st + dsp) % 2 == 0:
                            nc.scalar.copy(
                                o_sb[:, dsp * SPAN:(dsp + 1) * SPAN], o_ps[:])
                        else:
                            nc.vector.tensor_copy(
                                o_sb[:, dsp * SPAN:(dsp + 1) * SPAN], o_ps[:])
                    nc.sync.dma_start(out_d[s0:s0 + P, :], o_sb[:])

    nc.finalize()
    return nc


def kernel(resid, W_Q, W_K, W_V, W_out, b_out):
    global LAST_RESULTS, _CACHED_NC
    resid = np.asarray(resid, np.float32)
    W_Q = np.asarray(W_Q, np.float32)
    W_K = np.asarray(W_K, np.float32)
    W_V = np.asarray(W_V, np.float32)
    W_out = np.asarray(W_out, np.float32)
    b_out = np.asarray(b_out, np.float32)

    if _CACHED_NC is None:
        _CACHED_NC = _build_program()
    nc = _CACHED_NC

    residT = [np.ascontiguousarray(resid[b].T) for b in range(2)]
    in_maps = []
    for c in range(8):
        b, q = c // 4, c % 4
        # interleaved head order [h0, h2, h1, h3]: storage slot (g, i) holds
        # local head 2g+i -> qT[i]/zc[i] rows g*64 (see _build_program)
        heads = [4 * q, 4 * q + 2, 4 * q + 1, 4 * q + 3]
        groups = [2 * q, 2 * q + 1]
        in_maps.append({
            "resid_t": residT[b],
            "wq": np.ascontiguousarray(W_Q[:, heads, :].reshape(D, 256)),
            "wk": np.ascontiguousarray(W_K[:, groups, :].reshape(D, 128)),
            "wv": np.ascontiguousarray(W_V[:, groups, :].reshape(D, 128)),
            "wo": np.ascontiguousarray(
                W_out[:, heads, :].transpose(1, 0, 2).reshape(256, D)),
            "ones": np.ones((P, 1), np.float32),
        })

    res = run_bass_kernel_spmd(nc, in_maps, core_ids=list(range(8)))
    LAST_RESULTS = res

    out = np.zeros((2, S, D), np.float32)
    for c in range(8):
        out[c // 4] += res.results[c]["out"]
    out += b_out
    return out


# revision 18
# speedup vs baseline: 1.0073x; 1.0073x over previous
"""GQA attention kernel for Trainium2, 8 NeuronCores.

Problem: resid [2, 2048, 1024], 16 Q heads / 8 KV groups, d_head 64, causal,
out = softmax(QK^T/8 + causal) V -> W_out + b_out.

Sharding: tensor-parallel over (batch x kv-group-pairs). Core c handles
batch b = c // 4 and kv groups {2*(c%4), 2*(c%4)+1} = 4 Q heads. Each core
computes its heads' attention and a partial output projection; the host sums
the 4 partials per batch element and adds b_out.

Per-core dataflow (fp32 storage, float32r matmuls = full PE speed at
moving-dim >= 256):
  - host passes resid[b].T so the d_model contraction lands on partitions
  - Q^T [256, S] and K^T [128, S] projections (PSUM accum over 8 d-chunks)
  - V [S, 2x65] with a ones column appended per group -> the AV matmul
    produces sum-exp for free in output row 64
  - scores computed transposed: S^T[k, q] = K @ Q^T; causality via q-start
    offset, zero-padding of exp tiles, and an upper-triangular
    multiplicative mask on diagonal tiles
  - softmax without max-subtraction (scores are O(1) by construction;
    masked lanes are exactly zero after the mask multiply)
  - U^T[e, q] += V_aug^T @ exp accumulated over k-tiles in PSUM
  - normalize: reciprocal of row 64 (VectorE), partition-broadcast
    (GpSimd), multiply into z^T (VectorE)
  - out_partial[s, d] = z^T.T @ W_out_stack accumulated over 2 e-chunks
"""

import sys

sys.path.insert(0, "/opt/trn_rl_repo")

import numpy as np

import concourse.bass as bass
import concourse.mybir as mybir
import concourse.tile as tile
from concourse import bacc
from concourse.bass_utils import run_bass_kernel_spmd
from concourse.masks import make_upper_triangular

S = 2048          # seq len
D = 1024          # d_model
E = 64            # d_head
P = 128
NC_HEADS = 4      # heads per core
NCHUNK = D // P   # 8 d_model chunks
SPAN = 512
NSPAN = S // SPAN
NKT = S // P      # 16 k tiles
F32 = mybir.dt.float32
F32R = mybir.dt.float32r
EXP = mybir.ActivationFunctionType.Exp

LAST_RESULTS = None  # stashed BassKernelResults for the test harness
_CACHED_NC = None


def _build_program():
    nc = bacc.Bacc("TRN2", target_bir_lowering=False, debug=False)

    rT_d = nc.dram_tensor("resid_t", [D, S], F32R, kind="ExternalInput")
    wq_d = nc.dram_tensor("wq", [D, 256], F32R, kind="ExternalInput")
    wk_d = nc.dram_tensor("wk", [D, 128], F32R, kind="ExternalInput")
    wv_d = nc.dram_tensor("wv", [D, 128], F32R, kind="ExternalInput")
    wo_d = nc.dram_tensor("wo", [256, D], F32R, kind="ExternalInput")
    ones_d = nc.dram_tensor("ones", [P, 1], F32R, kind="ExternalInput")
    out_d = nc.dram_tensor("out", [S, D], F32, kind="ExternalOutput")

    with tile.TileContext(nc) as tc:
        with (
            tc.tile_pool(name="persist", bufs=1) as pp,
            tc.tile_pool(name="exp", bufs=6) as ep,
            tc.tile_pool(name="zt", bufs=2) as zp,
            tc.tile_pool(name="misc", bufs=4) as mp,
            tc.tile_pool(name="ostage", bufs=3) as op,
            tc.tile_pool(name="ps_u", bufs=4, space="PSUM") as ps_u,
            tc.tile_pool(name="ps_sc", bufs=3, space="PSUM") as ps_sc,
            tc.tile_pool(name="ps_op", bufs=1, space="PSUM") as ps_op,
        ):
            # ---- load weights + transposed residual ----
            wq_sb = []
            wk_sb = []
            wv_sb = []
            for c in range(NCHUNK):
                t = pp.tile([P, 256], F32R, tag=f"wq{c}")
                nc.sync.dma_start(t[:], wq_d[c * P:(c + 1) * P, :])
                wq_sb.append(t)
                t = pp.tile([P, 128], F32R, tag=f"wk{c}")
                nc.sync.dma_start(t[:], wk_d[c * P:(c + 1) * P, :])
                wk_sb.append(t)
                t = pp.tile([P, 128], F32R, tag=f"wv{c}")
                nc.sync.dma_start(t[:], wv_d[c * P:(c + 1) * P, :])
                wv_sb.append(t)
            wo_sb = []
            for c in range(2):
                t = pp.tile([P, D], F32R, tag=f"wo{c}")
                nc.sync.dma_start(t[:], wo_d[c * P:(c + 1) * P, :])
                wo_sb.append(t)

            mask = pp.tile([P, P], F32, tag="mask")
            make_upper_triangular(nc, mask[:], val=1.0, diag=True)

            # residual chunks, DMA'd span-wise so projection accumulation
            # groups (which need all 8 d-chunks of one span) start after
            # ~2MB instead of the full 8.4MB
            rT = []
            for c in range(NCHUNK):
                t = pp.tile([P, S], F32R, tag=f"rt{c}", name=f"rt{c}")
                rT.append(t)
            for sp in range(NSPAN):
                for c in range(NCHUNK):
                    nc.sync.dma_start(
                        rT[c][:, sp * SPAN:(sp + 1) * SPAN],
                        rT_d[c * P:(c + 1) * P, sp * SPAN:(sp + 1) * SPAN])

            # ---- Q^T projection: qT[eblk] [128, S], eblk 0 = heads 0,1 ----
            qT = []
            for eblk in range(2):
                qt = pp.tile([P, S], F32R, tag=f"qt{eblk}")
                qT.append(qt)
                for sp in range(NSPAN):
                    acc = ps_u.tile([P, SPAN], F32, tag="u")
                    for c in range(NCHUNK):
                        nc.tensor.matmul(
                            acc[:],
                            wq_sb[c][:, eblk * P:(eblk + 1) * P],
                            rT[c][:, sp * SPAN:(sp + 1) * SPAN],
                            start=(c == 0),
                            stop=(c == NCHUNK - 1),
                        )
                    nc.scalar.copy(qt[:, sp * SPAN:(sp + 1) * SPAN], acc[:])

            # ---- K^T projection: kT [128, S] (rows = 2 groups x 64) ----
            kT = pp.tile([P, S], F32R, tag="kt")
            for sp in range(NSPAN):
                acc = ps_u.tile([P, SPAN], F32, tag="u")
                for c in range(NCHUNK):
                    nc.tensor.matmul(
                        acc[:],
                        wk_sb[c][:],
                        rT[c][:, sp * SPAN:(sp + 1) * SPAN],
                        start=(c == 0),
                        stop=(c == NCHUNK - 1),
                    )
                nc.scalar.copy(kT[:, sp * SPAN:(sp + 1) * SPAN], acc[:])

            # ---- V projection + ones column: vaug[kt] [128, 130] ----
            vaug = []
            for kt in range(NKT):
                va = pp.tile([P, 130], F32R, tag=f"va{kt}")
                vaug.append(va)
                acc = ps_sc.tile([P, SPAN], F32, tag="sc")
                for c in range(NCHUNK):
                    nc.tensor.matmul(
                        acc[:, 0:128],
                        rT[c][:, kt * P:(kt + 1) * P],
                        wv_sb[c][:],
                        start=(c == 0),
                        stop=(c == NCHUNK - 1),
                    )
                nc.vector.tensor_copy(va[:, 0:64], acc[:, 0:64])
                nc.vector.tensor_copy(va[:, 65:129], acc[:, 64:128])
                nc.sync.dma_start(va[:, 64:65], ones_d[:])
                nc.sync.dma_start(va[:, 129:130], ones_d[:])

            # ---- attention + output projection, span by span ----
            for sp in range(NSPAN):
                q0 = sp * SPAN
                nkt = (q0 + SPAN) // P  # k tiles touching this span
                # head slot (g, i): local head 2g+i, stored in qT[i] rows
                # g*64:(g+1)*64 so scores lhsT/rhs share base partition g*64
                # (and g0/g1 matmuls row-pack the PE array).
                u_ps = [ps_u.tile([P, SPAN], F32, tag="u", name=f"u{j}")
                        for j in range(NC_HEADS)]
                for kt in range(nkt):
                    k0 = kt * P
                    off = max(k0 - q0, 0)
                    w = SPAN - off
                    for g in range(2):
                        for i in range(2):
                            s_ps = ps_sc.tile([P, SPAN], F32, tag="sc",
                                              name=f"s{g}{i}")
                            nc.tensor.matmul(
                                s_ps[:, off:off + w],
                                kT[g * 64:(g + 1) * 64, k0:k0 + P],
                                qT[i][g * 64:(g + 1) * 64,
                                         q0 + off:q0 + off + w],
                                start=True,
                                stop=True,
                            )
                            e_sb = ep.tile([P, SPAN], F32R, tag="e",
                                           name=f"e{g}{i}")
                            nc.scalar.activation(
                                e_sb[:, off:off + w], s_ps[:, off:off + w],
                                EXP, scale=0.125,
                            )
                            if k0 >= q0:  # diagonal tile -> causal mask
                                nc.vector.tensor_mul(
                                    e_sb[:, off:off + P],
                                    e_sb[:, off:off + P].bitcast(F32),
                                    mask[:],
                                )
                            # partial-width accumulate: cols < off keep
                            # earlier k-tiles' sums (kt==0 covers full span)
                            nc.tensor.matmul(
                                u_ps[2 * g + i][0:65, off:off + w],
                                vaug[kt][:, g * 65:(g + 1) * 65],
                                e_sb[:, off:off + w],
                                start=(kt == 0),
                                stop=(kt == nkt - 1),
                                skip_group_check=True,
                            )

                # normalize -> z^T chunks; zc[i] rows g*64 = head slot (g, i),
                # matching the host-side wo packing [h0, h2 | h1, h3]
                zc = [zp.tile([P, SPAN], F32R, tag=f"zt{c}", name=f"z{c}")
                      for c in range(2)]
                for g in range(2):
                    for i in range(2):
                        rec = mp.tile([1, SPAN], F32, tag="rec", name="rec")
                        nc.vector.reciprocal(rec[:], u_ps[2 * g + i][64:65, :])
                        bc = mp.tile([64, SPAN], F32, tag="bc", name="bc")
                        nc.gpsimd.partition_broadcast(bc[:], rec[:])
                        nc.vector.tensor_mul(
                            zc[i][g * 64:(g + 1) * 64, :],
                            u_ps[2 * g + i][0:64, :],
                            bc[:],
                        )

                # output projection for this span of s
                for st in range(4):
                    s0 = q0 + st * P
                    o_sb = op.tile([P, D], F32, tag="ost")
                    for dsp in range(2):
                        o_ps = ps_op.tile([P, SPAN], F32, tag="op")
                        for ch in range(2):
                            nc.tensor.matmul(
                                o_ps[:],
                                zc[ch][:, st * P:(st + 1) * P],
                                wo_sb[ch][:, dsp * SPAN:(dsp + 1) * SPAN],
                                start=(ch == 0),
                                stop=(ch == 1),
                            )
                        if (st + dsp) % 2 == 0:
                            nc.scalar.copy(
                                o_sb[:, dsp * SPAN:(dsp + 1) * SPAN], o_ps[:])
                        else:
                            nc.vector.tensor_copy(
                                o_sb[:, dsp * SPAN:(dsp + 1) * SPAN], o_ps[:])
                    nc.sync.dma_start(out_d[s0:s0 + P, :], o_sb[:])

    nc.finalize()
    return nc


def kernel(resid, W_Q, W_K, W_V, W_out, b_out):
    global LAST_RESULTS, _CACHED_NC
    resid = np.asarray(resid, np.float32)
    W_Q = np.asarray(W_Q, np.float32)
    W_K = np.asarray(W_K, np.float32)
    W_V = np.asarray(W_V, np.float32)
    W_out = np.asarray(W_out, np.float32)
    b_out = np.asarray(b_out, np.float32)

    if _CACHED_NC is None:
        _CACHED_NC = _build_program()
    nc = _CACHED_NC

    residT = [np.ascontiguousarray(resid[b].T) for b in range(2)]
    in_maps = []
    for c in range(8):
        b, q = c // 4, c % 4
        # interleaved head order [h0, h2, h1, h3]: storage slot (g, i) holds
        # local head 2g+i -> qT[i]/zc[i] rows g*64 (see _build_program)
        heads = [4 * q, 4 * q + 2, 4 * q + 1, 4 * q + 3]
        groups = [2 * q, 2 * q + 1]
        in_maps.append({
            "resid_t": residT[b],
            "wq": np.ascontiguousarray(W_Q[:, heads, :].reshape(D, 256)),
            "wk": np.ascontiguousarray(W_K[:, groups, :].reshape(D, 128)),
            "wv": np.ascontiguousarray(W_V[:, groups, :].reshape(D, 128)),
            "wo": np.ascontiguousarray(
                W_out[:, heads, :].transpose(1, 0, 2).reshape(256, D)),
            "ones": np.ones((P, 1), np.float32),
        })

    res = run_bass_kernel_spmd(nc, in_maps, core_ids=list(range(8)))
    LAST_RESULTS = res

    out = np.zeros((2, S, D), np.float32)
    for c in range(8):
        out[c // 4] += res.results[c]["out"]
    out += b_out
    return out
